# revision 21
# baseline (speedup 1.0000x reference)
"""Trainium2 Bass kernel for nn_AlpacaMoca_15109694948019.

Math: per (b,z,u) with A = Linv[b,z,u] (128x128), phi = encoder(x)[b]:
    mu   = phi^T A Q
    sig  = phi^T A phi
    pred = exp(logSigEps[u]) * (1 + sig)

Sharding: batch (B=16) split 2 rows per core across 8 cores. Linv/Q are
fully independent along b; encoder weights + logSigEps replicated.

The kernel is HBM-bound (Linv is 512 MiB in fp32). Key choices:
- Linv and Q are pre-cast to fp16 on the host: halves the HBM stream and
  enables PE fast-weight-load. Values are ~N(0, 0.02); costs ~3e-4
  relative error.
- The 32 MiB/core Linv stream is split across BOTH HWDGE rings so the
  two DMA paths run concurrently (~HBM limit):
    * even z-blocks: xbar DMA-transpose on the SP ring -> A^T tiles
      [p=partition, q=free]. One fp16 matmul per matrix: stationary A^T,
      moving [Q_j | phi] (N=2) -> PSUM [K_j | S_j] = [A Q | A phi] on
      q-partitions. (Concurrent transposes on BOTH rings corrupt data on
      HW — transposes stay on one ring only.)
    * odd z-blocks: direct loads on the ACT ring -> A tiles
      [q=partition, p=free]. One fp16 matmul per matrix: stationary A,
      moving phi (N=1) -> t_j = A^T phi on p-partitions.
- Reductions: mu/sig are per-column dots against phi (transposed half)
  or against Q^T/phi (direct half): DVE elementwise multiply then
  ones-vector fp32 matmul column sums, written per z-block straight into
  j-ordered PSUM slices.
"""

import numpy as np

B, Z, U, P, X, H = 16, 64, 8, 128, 64, 128
N_CORES = 8
B_PER = B // N_CORES          # 2 batch rows per core
J = Z * U                     # 512 matrices per batch row
ZT = 8                        # z rows per Linv DMA tile (2 MiB fp16)
NBLK = Z // ZT                # 8 z-blocks per batch row (64 matrices each)
BLKJ = ZT * U                 # 64 matrices per z-block
LINV_BUFS = 4                 # SBUF buffer depth per Linv stream

_CACHE = {}


def _build_nc():
    import concourse.bacc as bacc
    import concourse.tile as tile
    from concourse import mybir

    f32 = mybir.dt.float32
    f16 = mybir.dt.float16
    AF = mybir.ActivationFunctionType
    Alu = mybir.AluOpType

    nc = bacc.Bacc("TRN2")

    x_d = nc.dram_tensor("x", [B_PER, X], f32, kind="ExternalInput")
    linv_d = nc.dram_tensor("Linv", [B_PER, Z, U, P, P], f16, kind="ExternalInput")
    q_d = nc.dram_tensor("Q", [B_PER, Z, U, P], f16, kind="ExternalInput")
    w1_d = nc.dram_tensor("W1", [X, H], f32, kind="ExternalInput")
    b1_d = nc.dram_tensor("b1", [H], f32, kind="ExternalInput")
    w2_d = nc.dram_tensor("W2", [H, H], f32, kind="ExternalInput")
    b2_d = nc.dram_tensor("b2", [H], f32, kind="ExternalInput")
    w3_d = nc.dram_tensor("W3", [H, H], f32, kind="ExternalInput")
    b3_d = nc.dram_tensor("b3", [H], f32, kind="ExternalInput")
    w4_d = nc.dram_tensor("W4", [H, P], f32, kind="ExternalInput")
    b4_d = nc.dram_tensor("b4", [P], f32, kind="ExternalInput")
    lse_d = nc.dram_tensor("logSigEps", [U], f32, kind="ExternalInput")

    mu_d = nc.dram_tensor("mu", [B_PER, J], f32, kind="ExternalOutput")
    pred_d = nc.dram_tensor("pred", [B_PER, J], f32, kind="ExternalOutput")

    with tile.TileContext(nc) as tc:
        with (
            tc.tile_pool(name="const", bufs=1) as cpool,
            tc.tile_pool(name="lin", bufs=LINV_BUFS) as lpool,
            tc.tile_pool(name="work", bufs=2) as wpool,
            tc.tile_pool(name="outp", bufs=2) as opool,
            tc.tile_pool(name="encps", bufs=2, space="PSUM") as encps,
            tc.tile_pool(name="tps", bufs=2, space="PSUM") as tpool,
            tc.tile_pool(name="mvps", bufs=1, space="PSUM") as mvpool,
        ):
            # ---- constants (small loads on SWDGE to keep HWDGE rings free) ----
            ones = cpool.tile([128, 1], f32)
            nc.gpsimd.memset(ones[:], 1.0)

            w1 = cpool.tile([X, H], f32)
            nc.gpsimd.dma_start(w1[:], w1_d[:])
            w2 = cpool.tile([H, H], f32)
            nc.gpsimd.dma_start(w2[:], w2_d[:])
            w3 = cpool.tile([H, H], f32)
            nc.gpsimd.dma_start(w3[:], w3_d[:])
            w4 = cpool.tile([H, P], f32)
            nc.gpsimd.dma_start(w4[:], w4_d[:])
            b1 = cpool.tile([H, 1], f32)
            nc.gpsimd.dma_start(b1[:], b1_d[:, None])
            b2 = cpool.tile([H, 1], f32)
            nc.gpsimd.dma_start(b2[:], b2_d[:, None])
            b3 = cpool.tile([H, 1], f32)
            nc.gpsimd.dma_start(b3[:], b3_d[:, None])
            b4 = cpool.tile([P, 1], f32)
            nc.gpsimd.dma_start(b4[:], b4_d[:, None])
            xT = cpool.tile([X, B_PER], f32)
            nc.gpsimd.dma_start(xT[:], x_d[:].rearrange("b x -> x b"))
            lse = cpool.tile([1, U], f32)
            nc.gpsimd.dma_start(lse[:], lse_d[None, :])

            # exp(logSigEps) replicated 64x along free dim -> [1, J] (u fastest)
            esig = cpool.tile([1, J], f32)
            nc.scalar.activation(esig[:, 0:U], lse[:], AF.Exp)
            n = U
            while n < J:
                m = min(n, J - n)
                nc.vector.tensor_copy(esig[:, n : n + m], esig[:, 0:m])
                n += m

            # ---- encoder: phi^T as [P, B_PER] ----
            def elu(h_ps, bias, out_sb):
                # out = elu(h_ps + bias) = relu(z) + exp(min(z,0)) - 1
                r = wpool.tile([H, B_PER], f32, tag="elu_r")
                nc.scalar.activation(r[:], h_ps[:], AF.Relu, bias=bias[:])
                zm = wpool.tile([H, B_PER], f32, tag="elu_z")
                nc.scalar.activation(zm[:], h_ps[:], AF.Identity, bias=bias[:])
                nc.vector.tensor_scalar_min(zm[:], zm[:], 0.0)
                nc.scalar.activation(zm[:], zm[:], AF.Exp)
                nc.vector.scalar_tensor_tensor(
                    out=out_sb[:], in0=zm[:], scalar=-1.0, in1=r[:],
                    op0=Alu.add, op1=Alu.add,
                )

            hp = encps.tile([H, B_PER], f32, tag="encmm")
            nc.tensor.matmul(hp[:], w1[:], xT[:])
            h1 = wpool.tile([H, B_PER], f32, tag="h")
            elu(hp, b1, h1)

            hp = encps.tile([H, B_PER], f32, tag="encmm")
            nc.tensor.matmul(hp[:], w2[:], h1[:])
            h2 = wpool.tile([H, B_PER], f32, tag="h")
            elu(hp, b2, h2)

            hp = encps.tile([H, B_PER], f32, tag="encmm")
            nc.tensor.matmul(hp[:], w3[:], h2[:])
            h3 = wpool.tile([H, B_PER], f32, tag="h")
            elu(hp, b3, h3)

            hp = encps.tile([P, B_PER], f32, tag="encmm")
            nc.tensor.matmul(hp[:], w4[:], h3[:])
            phi32 = cpool.tile([P, B_PER], f32)
            nc.scalar.activation(phi32[:], hp[:], AF.Identity, bias=b4[:])
            phi16 = cpool.tile([P, B_PER], f16)
            nc.vector.tensor_copy(phi16[:], phi32[:])

            # ---- per b: Q^T (fp16 + f32) and QP = [Q_j | phi] fp16 ----
            QPs, qT32s = [], []
            for bi in range(B_PER):
                qT16 = wpool.tile([P, J], f16, tag="qT16")
                nc.sync.dma_start(
                    qT16[:], q_d[bi].rearrange("z u p -> (z u) p"), transpose=True
                )
                qT32 = wpool.tile([P, J], f32, tag="qT32")
                nc.vector.tensor_copy(qT32[:], qT16[:])
                QP = wpool.tile([P, J, 2], f16, tag="QP")
                nc.vector.tensor_copy(QP[:, :, 0], qT16[:])
                # broadcast phi along free: Identity(0*in + bias[p])
                nc.scalar.activation(
                    QP[:, :, 1], qT16[:], AF.Identity,
                    bias=phi32[:, bi : bi + 1], scale=0.0,
                )
                QPs.append(QP)
                qT32s.append(qT32)

            # ---- main loop ----
            for bi in range(B_PER):
                # transposed half (even z-blocks): [K_j | S_j] on q-partitions
                T2_ps = tpool.tile([P, (NBLK // 2) * BLKJ, 2], f32, tag="T2")
                # direct half (odd z-blocks): t_j on p-partitions
                T1_ps = tpool.tile([P, (NBLK // 2) * BLKJ], f32, tag="T1")

                for zt in range(NBLK):
                    blk = zt // 2
                    if zt % 2 == 0:
                        ltT = lpool.tile([128, BLKJ * P], f16, tag="linvT")
                        src = linv_d[bi, zt * ZT : (zt + 1) * ZT].rearrange(
                            "z u q p -> (z u q) p"
                        )
                        nc.sync.dma_start(ltT[:], src, transpose=True)
                        for m in range(BLKJ):
                            jj = zt * BLKJ + m
                            nc.tensor.matmul(
                                T2_ps[:, blk * BLKJ + m, :],
                                ltT[:, m * P : (m + 1) * P],
                                QPs[bi][:, jj, :],
                            )
                    else:
                        lt = lpool.tile([128, BLKJ, P], f16, tag="linvD")
                        src = linv_d[bi, zt * ZT : (zt + 1) * ZT].rearrange(
                            "z u q p -> q (z u) p"
                        )
                        nc.scalar.dma_start(lt[:], src)
                        for m in range(BLKJ):
                            c = blk * BLKJ + m
                            nc.tensor.matmul(
                                T1_ps[:, c : c + 1],
                                lt[:, m, :],
                                phi16[:, bi : bi + 1],
                            )

                # ---- reductions into j-ordered PSUM slices ----
                mv_ps = mvpool.tile([1, 2 * J], f32, tag="mv")

                VW2 = wpool.tile([P, (NBLK // 2) * BLKJ, 2], f32, tag="VW2")
                nc.vector.tensor_scalar_mul(VW2[:], T2_ps[:], phi32[:, bi : bi + 1])
                V1 = wpool.tile([P, (NBLK // 2) * BLKJ], f32, tag="V1")
                nc.vector.tensor_scalar_mul(V1[:], T1_ps[:], phi32[:, bi : bi + 1])
                U1 = wpool.tile([P, (NBLK // 2) * BLKJ], f32, tag="U1")

                for zt in range(NBLK):
                    blk = zt // 2
                    c0, c1 = blk * BLKJ, (blk + 1) * BLKJ
                    j0, j1 = zt * BLKJ, (zt + 1) * BLKJ
                    if zt % 2 == 0:
                        nc.tensor.matmul(
                            mv_ps[:, j0:j1], ones[:], VW2[:, c0:c1, 0]
                        )
                        nc.tensor.matmul(
                            mv_ps[:, J + j0 : J + j1], ones[:], VW2[:, c0:c1, 1]
                        )
                    else:
                        nc.vector.tensor_tensor(
                            U1[:, c0:c1], T1_ps[:, c0:c1], qT32s[bi][:, j0:j1],
                            op=Alu.mult,
                        )
                        nc.tensor.matmul(mv_ps[:, j0:j1], ones[:], U1[:, c0:c1])
                        nc.tensor.matmul(
                            mv_ps[:, J + j0 : J + j1], ones[:], V1[:, c0:c1]
                        )

                mu_sb = opool.tile([1, J], f32, tag="mu")
                nc.vector.tensor_copy(mu_sb[:], mv_ps[:, 0:J])
                pr_sb = opool.tile([1, J], f32, tag="pr")
                # pred = esig * (1 + sig)
                nc.vector.tensor_scalar_add(pr_sb[:], mv_ps[:, J : 2 * J], 1.0)
                nc.vector.tensor_mul(pr_sb[:], pr_sb[:], esig[:])

                nc.sync.dma_start(mu_d[bi : bi + 1, :], mu_sb[:])
                nc.sync.dma_start(pred_d[bi : bi + 1, :], pr_sb[:])

    nc.finalize()
    return nc


def _get_nc():
    if "nc" not in _CACHE:
        _CACHE["nc"] = _build_nc()
    return _CACHE["nc"]


def _make_in_maps(inputs):
    x = np.ascontiguousarray(np.asarray(inputs["x"], dtype=np.float32))
    Linv = np.ascontiguousarray(
        np.asarray(inputs["Linv"], dtype=np.float32).astype(np.float16)
    )
    Q2 = np.ascontiguousarray(
        np.asarray(inputs["Q"], dtype=np.float32)[:, :, :, 0, :].astype(np.float16)
    )
    shared = {
        n: np.ascontiguousarray(np.asarray(inputs[n], np.float32))
        for n in ["W1", "b1", "W2", "b2", "W3", "b3", "W4", "b4", "logSigEps"]
    }
    in_maps = []
    for c in range(N_CORES):
        sl = slice(c * B_PER, (c + 1) * B_PER)
        in_maps.append({"x": x[sl], "Linv": Linv[sl], "Q": Q2[sl], **shared})
    return in_maps


def kernel(x, Linv, Q, W1, b1, W2, b2, W3, b3, W4, b4, logSigEps):
    from concourse.bass_utils import run_bass_kernel_spmd

    in_maps = _make_in_maps(dict(
        x=x, Linv=Linv, Q=Q, W1=W1, b1=b1, W2=W2, b2=b2, W3=W3, b3=b3,
        W4=W4, b4=b4, logSigEps=logSigEps,
    ))
    nc = _get_nc()
    res = run_bass_kernel_spmd(nc, in_maps, list(range(N_CORES))).results

    mu = np.concatenate([r["mu"] for r in res], axis=0).reshape(B, Z, U, 1)
    pred = np.concatenate([r["pred"] for r in res], axis=0).reshape(B, Z, U)
    return mu, pred


# revision 22
# speedup vs baseline: 1.0804x; 1.0804x over previous
"""Trainium2 Bass kernel for nn_AlpacaMoca_15109694948019.

Math: per (b,z,u) with A = Linv[b,z,u] (128x128), phi = encoder(x)[b]:
    mu   = phi^T A Q
    sig  = phi^T A phi
    pred = exp(logSigEps[u]) * (1 + sig)

Sharding: batch (B=16) split 2 rows per core across 8 cores. Linv/Q are
fully independent along b; encoder weights + logSigEps replicated.

The kernel is HBM-bound (Linv is 512 MiB in fp32). Key choices:
- Linv and Q are pre-cast to fp16 on the host: halves the HBM stream and
  enables PE fast-weight-load. Values are ~N(0, 0.02); costs ~3e-4
  relative error.
- The 32 MiB/core Linv stream is split across BOTH HWDGE rings so the
  two DMA paths run concurrently (~HBM limit):
    * even z-blocks: xbar DMA-transpose on the SP ring -> A^T tiles
      [p=partition, q=free]. One fp16 matmul per matrix: stationary A^T,
      moving [Q_j | phi] (N=2) -> PSUM [K_j | S_j] = [A Q | A phi] on
      q-partitions. (Concurrent transposes on BOTH rings corrupt data on
      HW — transposes stay on one ring only.)
    * odd z-blocks: direct loads on the ACT ring -> A tiles
      [q=partition, p=free]. One fp16 matmul per matrix: stationary A,
      moving phi (N=1) -> t_j = A^T phi on p-partitions.
- Reductions: mu/sig are per-column dots against phi (transposed half)
  or against Q^T/phi (direct half): DVE elementwise multiply then
  ones-vector fp32 matmul column sums, written per z-block straight into
  j-ordered PSUM slices.
"""

import numpy as np

B, Z, U, P, X, H = 16, 64, 8, 128, 64, 128
N_CORES = 8
B_PER = B // N_CORES          # 2 batch rows per core
J = Z * U                     # 512 matrices per batch row
ZT = 8                        # z rows per Linv DMA tile (2 MiB fp16)
NBLK = Z // ZT                # 8 z-blocks per batch row (64 matrices each)
BLKJ = ZT * U                 # 64 matrices per z-block
LINV_BUFS = 4                 # SBUF buffer depth per Linv stream

_CACHE = {}


def _build_nc():
    import concourse.bacc as bacc
    import concourse.tile as tile
    from concourse import mybir

    f32 = mybir.dt.float32
    f16 = mybir.dt.float16
    AF = mybir.ActivationFunctionType
    Alu = mybir.AluOpType

    nc = bacc.Bacc("TRN2")

    x_d = nc.dram_tensor("x", [B_PER, X], f32, kind="ExternalInput")
    linv_d = nc.dram_tensor("Linv", [B_PER, Z, U, P, P], f16, kind="ExternalInput")
    q_d = nc.dram_tensor("Q", [B_PER, Z, U, P], f16, kind="ExternalInput")
    w1_d = nc.dram_tensor("W1", [X, H], f32, kind="ExternalInput")
    b1_d = nc.dram_tensor("b1", [H], f32, kind="ExternalInput")
    w2_d = nc.dram_tensor("W2", [H, H], f32, kind="ExternalInput")
    b2_d = nc.dram_tensor("b2", [H], f32, kind="ExternalInput")
    w3_d = nc.dram_tensor("W3", [H, H], f32, kind="ExternalInput")
    b3_d = nc.dram_tensor("b3", [H], f32, kind="ExternalInput")
    w4_d = nc.dram_tensor("W4", [H, P], f32, kind="ExternalInput")
    b4_d = nc.dram_tensor("b4", [P], f32, kind="ExternalInput")
    lse_d = nc.dram_tensor("logSigEps", [U], f32, kind="ExternalInput")

    mu_d = nc.dram_tensor("mu", [B_PER, J], f32, kind="ExternalOutput")
    pred_d = nc.dram_tensor("pred", [B_PER, J], f32, kind="ExternalOutput")

    with tile.TileContext(nc) as tc:
        with (
            tc.tile_pool(name="const", bufs=1) as cpool,
            tc.tile_pool(name="lin", bufs=LINV_BUFS) as lpool,
            tc.tile_pool(name="work", bufs=2) as wpool,
            tc.tile_pool(name="outp", bufs=2) as opool,
            tc.tile_pool(name="encps", bufs=2, space="PSUM") as encps,
            tc.tile_pool(name="tps", bufs=2, space="PSUM") as tpool,
            tc.tile_pool(name="mvps", bufs=1, space="PSUM") as mvpool,
        ):
            # ---- constants (SP ring: gpsimd/SWDGE starts ~20us late) ----
            ones = cpool.tile([128, 1], f32)
            nc.gpsimd.memset(ones[:], 1.0)

            w1 = cpool.tile([X, H], f32)
            nc.sync.dma_start(w1[:], w1_d[:])
            w2 = cpool.tile([H, H], f32)
            nc.sync.dma_start(w2[:], w2_d[:])
            w3 = cpool.tile([H, H], f32)
            nc.sync.dma_start(w3[:], w3_d[:])
            w4 = cpool.tile([H, P], f32)
            nc.sync.dma_start(w4[:], w4_d[:])
            b1 = cpool.tile([H, 1], f32)
            nc.sync.dma_start(b1[:], b1_d[:, None])
            b2 = cpool.tile([H, 1], f32)
            nc.sync.dma_start(b2[:], b2_d[:, None])
            b3 = cpool.tile([H, 1], f32)
            nc.sync.dma_start(b3[:], b3_d[:, None])
            b4 = cpool.tile([P, 1], f32)
            nc.sync.dma_start(b4[:], b4_d[:, None])
            xT = cpool.tile([X, B_PER], f32)
            nc.sync.dma_start(xT[:], x_d[:].rearrange("b x -> x b"))
            lse = cpool.tile([1, U], f32)
            nc.sync.dma_start(lse[:], lse_d[None, :])


            # ---- encoder: phi^T as [P, B_PER] ----
            def elu(h_ps, bias, out_sb):
                # out = elu(h_ps + bias) = relu(z) + exp(min(z,0)) - 1
                r = wpool.tile([H, B_PER], f32, tag="elu_r")
                nc.scalar.activation(r[:], h_ps[:], AF.Relu, bias=bias[:])
                zm = wpool.tile([H, B_PER], f32, tag="elu_z")
                nc.scalar.activation(zm[:], h_ps[:], AF.Identity, bias=bias[:])
                nc.vector.tensor_scalar_min(zm[:], zm[:], 0.0)
                nc.scalar.activation(zm[:], zm[:], AF.Exp)
                nc.vector.scalar_tensor_tensor(
                    out=out_sb[:], in0=zm[:], scalar=-1.0, in1=r[:],
                    op0=Alu.add, op1=Alu.add,
                )

            hp = encps.tile([H, B_PER], f32, tag="encmm")
            nc.tensor.matmul(hp[:], w1[:], xT[:])
            h1 = wpool.tile([H, B_PER], f32, tag="h")
            elu(hp, b1, h1)

            hp = encps.tile([H, B_PER], f32, tag="encmm")
            nc.tensor.matmul(hp[:], w2[:], h1[:])
            h2 = wpool.tile([H, B_PER], f32, tag="h")
            elu(hp, b2, h2)

            hp = encps.tile([H, B_PER], f32, tag="encmm")
            nc.tensor.matmul(hp[:], w3[:], h2[:])
            h3 = wpool.tile([H, B_PER], f32, tag="h")
            elu(hp, b3, h3)

            hp = encps.tile([P, B_PER], f32, tag="encmm")
            nc.tensor.matmul(hp[:], w4[:], h3[:])
            phi32 = cpool.tile([P, B_PER], f32)
            nc.scalar.activation(phi32[:], hp[:], AF.Identity, bias=b4[:])
            phi16 = cpool.tile([P, B_PER], f16)
            nc.vector.tensor_copy(phi16[:], phi32[:])

            # ---- per b: Q^T (fp16 + f32) and QP = [Q_j | phi] fp16 ----
            QPs, qT32s = [], []
            for bi in range(B_PER):
                qT16 = wpool.tile([P, J], f16, tag="qT16")
                nc.sync.dma_start(
                    qT16[:], q_d[bi].rearrange("z u p -> (z u) p"), transpose=True
                )
                qT32 = wpool.tile([P, J], f32, tag="qT32")
                nc.vector.tensor_copy(qT32[:], qT16[:])
                QP = wpool.tile([P, J, 2], f16, tag="QP")
                nc.vector.tensor_copy(QP[:, :, 0], qT16[:])
                # broadcast phi along free on DVE: (in0 * 0) + phi[p]
                nc.vector.tensor_scalar(
                    QP[:, :, 1], qT16[:], 0.0, phi32[:, bi : bi + 1],
                    op0=Alu.mult, op1=Alu.add,
                )
                QPs.append(QP)
                qT32s.append(qT32)


            # exp(logSigEps) replicated 64x along free dim -> [1, J] (u fastest)
            esig = cpool.tile([1, J], f32)
            nc.scalar.activation(esig[:, 0:U], lse[:], AF.Exp)
            n = U
            while n < J:
                m = min(n, J - n)
                nc.vector.tensor_copy(esig[:, n : n + m], esig[:, 0:m])
                n += m

            # ---- main loop ----
            for bi in range(B_PER):
                # transposed half (even z-blocks): [K_j | S_j] on q-partitions
                T2_ps = tpool.tile([P, (NBLK // 2) * BLKJ, 2], f32, tag="T2")
                # direct half (odd z-blocks): t_j on p-partitions
                T1_ps = tpool.tile([P, (NBLK // 2) * BLKJ], f32, tag="T1")

                for zt in range(NBLK):
                    blk = zt // 2
                    if zt % 2 == 0:
                        ltT = lpool.tile([128, BLKJ * P], f16, tag="linvT")
                        src = linv_d[bi, zt * ZT : (zt + 1) * ZT].rearrange(
                            "z u q p -> (z u q) p"
                        )
                        nc.sync.dma_start(ltT[:], src, transpose=True)
                        for m in range(BLKJ):
                            jj = zt * BLKJ + m
                            nc.tensor.matmul(
                                T2_ps[:, blk * BLKJ + m, :],
                                ltT[:, m * P : (m + 1) * P],
                                QPs[bi][:, jj, :],
                            )
                    else:
                        lt = lpool.tile([128, BLKJ, P], f16, tag="linvD")
                        src = linv_d[bi, zt * ZT : (zt + 1) * ZT].rearrange(
                            "z u q p -> q (z u) p"
                        )
                        nc.scalar.dma_start(lt[:], src)
                        for m in range(BLKJ):
                            c = blk * BLKJ + m
                            nc.tensor.matmul(
                                T1_ps[:, c : c + 1],
                                lt[:, m, :],
                                phi16[:, bi : bi + 1],
                            )

                # ---- reductions into j-ordered PSUM slices ----
                mv_ps = mvpool.tile([1, 2 * J], f32, tag="mv")

                VW2 = wpool.tile([P, (NBLK // 2) * BLKJ, 2], f32, tag="VW2")
                nc.vector.tensor_scalar_mul(VW2[:], T2_ps[:], phi32[:, bi : bi + 1])
                V1 = wpool.tile([P, (NBLK // 2) * BLKJ], f32, tag="V1")
                nc.vector.tensor_scalar_mul(V1[:], T1_ps[:], phi32[:, bi : bi + 1])
                U1 = wpool.tile([P, (NBLK // 2) * BLKJ], f32, tag="U1")

                for zt in range(NBLK):
                    blk = zt // 2
                    c0, c1 = blk * BLKJ, (blk + 1) * BLKJ
                    j0, j1 = zt * BLKJ, (zt + 1) * BLKJ
                    if zt % 2 == 0:
                        nc.tensor.matmul(
                            mv_ps[:, j0:j1], ones[:], VW2[:, c0:c1, 0]
                        )
                        nc.tensor.matmul(
                            mv_ps[:, J + j0 : J + j1], ones[:], VW2[:, c0:c1, 1]
                        )
                    else:
                        nc.vector.tensor_tensor(
                            U1[:, c0:c1], T1_ps[:, c0:c1], qT32s[bi][:, j0:j1],
                            op=Alu.mult,
                        )
                        nc.tensor.matmul(mv_ps[:, j0:j1], ones[:], U1[:, c0:c1])
                        nc.tensor.matmul(
                            mv_ps[:, J + j0 : J + j1], ones[:], V1[:, c0:c1]
                        )

                mu_sb = opool.tile([1, J], f32, tag="mu")
                nc.vector.tensor_copy(mu_sb[:], mv_ps[:, 0:J])
                pr_sb = opool.tile([1, J], f32, tag="pr")
                # pred = esig * (1 + sig)
                nc.vector.tensor_scalar_add(pr_sb[:], mv_ps[:, J : 2 * J], 1.0)
                nc.vector.tensor_mul(pr_sb[:], pr_sb[:], esig[:])

                nc.sync.dma_start(mu_d[bi : bi + 1, :], mu_sb[:])
                nc.sync.dma_start(pred_d[bi : bi + 1, :], pr_sb[:])

    nc.finalize()
    return nc


def _get_nc():
    if "nc" not in _CACHE:
        _CACHE["nc"] = _build_nc()
    return _CACHE["nc"]


def _make_in_maps(inputs):
    x = np.ascontiguousarray(np.asarray(inputs["x"], dtype=np.float32))
    Linv = np.ascontiguousarray(
        np.asarray(inputs["Linv"], dtype=np.float32).astype(np.float16)
    )
    Q2 = np.ascontiguousarray(
        np.asarray(inputs["Q"], dtype=np.float32)[:, :, :, 0, :].astype(np.float16)
    )
    shared = {
        n: np.ascontiguousarray(np.asarray(inputs[n], np.float32))
        for n in ["W1", "b1", "W2", "b2", "W3", "b3", "W4", "b4", "logSigEps"]
    }
    in_maps = []
    for c in range(N_CORES):
        sl = slice(c * B_PER, (c + 1) * B_PER)
        in_maps.append({"x": x[sl], "Linv": Linv[sl], "Q": Q2[sl], **shared})
    return in_maps


def kernel(x, Linv, Q, W1, b1, W2, b2, W3, b3, W4, b4, logSigEps):
    from concourse.bass_utils import run_bass_kernel_spmd

    in_maps = _make_in_maps(dict(
        x=x, Linv=Linv, Q=Q, W1=W1, b1=b1, W2=W2, b2=b2, W3=W3, b3=b3,
        W4=W4, b4=b4, logSigEps=logSigEps,
    ))
    nc = _get_nc()
    res = run_bass_kernel_spmd(nc, in_maps, list(range(N_CORES))).results

    mu = np.concatenate([r["mu"] for r in res], axis=0).reshape(B, Z, U, 1)
    pred = np.concatenate([r["pred"] for r in res], axis=0).reshape(B, Z, U)
    return mu, pred
